# revision 24
# baseline (speedup 1.0000x reference)
"""Trainium2 Bass kernel for nn_Avey (retrieval-knn block transformer).

Sharding: 8 cores; core c handles batch b=c//4, chunks i0=2*(c%4), i0+1.
Each core is fully independent (no collectives):
  - host ships E-major embeddings (f32) + cosine-normalized bf16 copy
  - retrieval scores for all (i,j) chunk pairs of its batch (bf16 GEMMs,
    free-dim max; replicated across the 4 cores of a batch so the SPMD
    program is uniform)
  - top-k selection via vector ops, weighted chunk-select via dynamic slices
  - 4 block layers (bf16 GEMMs, fp32 residual/stats, triangular masked
    cosine-sim attention)
  - logits GEMM over the full vocab for its 512 output tokens (bf16 out)
Host side does layout prep of constant weights (transpose/cast/fold) and
the embedding gather/normalize.
"""
import sys
import os

sys.path.insert(0, "/opt/trn_rl_repo")

import numpy as np
import ml_dtypes

import concourse.bass as bass
import concourse.bacc as bacc
import concourse.mybir as mybir
import concourse.tile as tile
from concourse.bass import ds, ts
from concourse.bass_utils import run_bass_kernel_spmd
from concourse.masks import make_identity

P = 128
V, E, L = 32000, 768, 4
C, TL = 256, 1024
ED = 3072
PROJ_IN = 2304
B, T = 2, 2048
N = T // C  # 8 chunks per batch
EK = E // P  # 6
EDK = ED // P  # 24
PK = PROJ_IN // P  # 18
TLK = TL // P  # 8
F32 = mybir.dt.float32
BF16 = mybir.dt.bfloat16
I32 = mybir.dt.int32
AF = mybir.ActivationFunctionType
OP = mybir.AluOpType
AX = mybir.AxisListType

NEG = -1.0e30
LDWOPT = bool(int(os.environ.get("AVEY_LDWOPT", "1")))


def _phase_a(nc, tc, Hs, ident_f, ones_row, dd):
    """Scores + selection + extended-H build (from host-shipped embeddings)."""
    with (
        tc.tile_pool(name="bigA", bufs=1) as bigA,
        tc.tile_pool(name="workA", bufs=2) as work,
        tc.tile_pool(name="smallA", bufs=2) as small,
        tc.tile_pool(name="psumA", bufs=5, space="PSUM") as psum,
        tc.tile_pool(name="psumAS", bufs=1, space="PSUM") as psumS,
        tc.tile_pool(name="psumACC", bufs=1, space="PSUM") as psumACC,
    ):
        xnb = bigA.tile([P, EK, T], BF16, tag="xnb")
        for g in range(4):  # split so early chunks land first
            nc.sync.dma_start(
                xnb[:, :, ds(g * 512, 512)], dd["xnb"][:, :, ds(g * 512, 512)]
            )
        xe = bigA.tile([P, EK, T], F32, tag="xe")
        nc.sync.dma_start(xe[:], dd["xe"][:])

        # sel8[:, i] = ones in column i (same on every partition)
        sel8 = small.tile([P, N, N], F32, tag="sel8")
        nc.vector.memset(sel8[:], 0.0)
        for i in range(N):
            nc.vector.memset(sel8[:, i, i : i + 1], 1.0)

        # ---- scores: for each chunk i, cos-sims vs all candidate cols ----
        # row i of pss accumulates scores for chunk i via one-hot lhsT;
        # the pss MMs are deferred so the PE FIFO never waits on reduces
        pss = psumACC.tile([N, N], F32, tag="pss")
        rmts = []
        for i in range(1, N):
            nb = (i * C + 511) // 512  # 512-wide candidate blocks
            rmt = work.tile([P, 2, N], F32, tag=f"rmt{i}", name=f"rmt{i}")
            nc.vector.memset(rmt[:], NEG / 512.0)
            rmts.append(rmt)
            for ci in range(2):
                pgs = []
                for b in range(nb):
                    cw = min(512, i * C - b * 512)
                    pg = psum.tile([P, 512], F32, tag="mm", name=f"pg{b}")
                    pgs.append((pg, cw))
                for k in range(EK):
                    for b, (pg, cw) in enumerate(pgs):
                        nc.tensor.matmul(
                            pg[:, :cw],
                            xnb[:, k, ds(i * C + ci * P, P)],
                            xnb[:, k, ds(b * 512, cw)],
                            start=(k == 0),
                            stop=(k == EK - 1),
                        )
                for b, (pg, cw) in enumerate(pgs):
                    nj = cw // C  # 1 or 2 chunks in this block
                    nc.vector.tensor_reduce(
                        rmt[:, ci, ds(2 * b, nj)],
                        pg[:, :cw].rearrange("p (j c) -> p j c", c=C),
                        AX.X,
                        OP.max,
                    )
        for i in range(1, N):
            for ci in range(2):
                nc.tensor.matmul(
                    pss[:],
                    sel8[:, i],
                    rmts[i - 1][:, ci],
                    start=(i == 1 and ci == 0),
                    stop=(i == N - 1 and ci == 1),
                )

        # ---- selection math on [N, N] rows ----
        srows16 = small.tile([N, 2 * N], F32, tag="srows")
        nc.vector.memset(srows16[:], NEG)
        nc.vector.tensor_copy(srows16[:, :N], pss[:])
        srows = srows16[:, :N]
        nselcol = small.tile([N, N], F32, tag="nselcol")
        nselrow = small.tile([N, 1], F32, tag="nselrow")
        iota8 = small.tile([N, N], F32, tag="iota8")
        nc.sync.dma_start(nselcol[:], dd["nselcol"][:])
        nc.sync.dma_start(nselrow[:], dd["nselrow"][:])
        nc.sync.dma_start(iota8[:], dd["iota8"][:])

        maxv = small.tile([N, 8], F32, tag="maxv")
        nc.vector.max(maxv[:], srows16[:])
        kth = small.tile([N, 1], F32, tag="kth")
        scr8 = small.tile([N, N], F32, tag="scr8")
        nc.vector.tensor_tensor(scr8[:], maxv[:], nselcol[:], OP.mult)
        nc.vector.tensor_reduce(kth[:], scr8[:], AX.X, OP.add)
        mask = small.tile([N, N], F32, tag="mask")
        nc.vector.tensor_scalar(mask[:], srows, kth[:], None, op0=OP.is_ge)
        # cumsum over 8 via 3 shift-adds (ping-pong)
        cumA = small.tile([N, N], F32, tag="cumA")
        cumB = small.tile([N, N], F32, tag="cumB")
        nc.vector.tensor_copy(cumA[:], mask[:])
        pairs = ((cumA, cumB), (cumB, cumA), (cumA, cumB))
        for sh, (src, dst) in zip((1, 2, 4), pairs):
            nc.vector.tensor_copy(dst[:, :sh], src[:, :sh])
            nc.vector.tensor_tensor(dst[:, sh:], src[:, sh:], src[:, : N - sh], OP.add)
        cum = cumB
        # first selected: fs = mask * (cum == 1); w = srows / (s_first + 1e-8)
        fs = small.tile([N, N], F32, tag="fs")
        nc.vector.tensor_scalar(fs[:], cum[:], 1.0, None, op0=OP.is_equal)
        nc.vector.tensor_tensor(fs[:], fs[:], mask[:], OP.mult)
        s_first = small.tile([N, 1], F32, tag="sfirst")
        nc.vector.tensor_tensor(scr8[:], fs[:], srows, OP.mult)
        nc.vector.tensor_reduce(s_first[:], scr8[:], AX.X, OP.add)
        nc.vector.tensor_scalar_add(s_first[:], s_first[:], 1.0e-8)
        nc.vector.reciprocal(s_first[:], s_first[:])
        wv = small.tile([N, N], F32, tag="wv")
        nc.vector.tensor_scalar_mul(wv[:], srows, s_first[:])
        # slotv = cum + (2 - n_sel)
        slotv = small.tile([N, N], F32, tag="slotv")
        nc.vector.tensor_scalar(slotv[:], cum[:], nselrow[:], None, op0=OP.add)
        # per-slot weight / source index  [N, 4]
        wslot = small.tile([N, 4], F32, tag="wslot")
        jslot = small.tile([N, 4], F32, tag="jslot")
        nc.vector.memset(wslot[:], 0.0)
        nc.vector.memset(jslot[:], 0.0)
        for s in range(3):
            sel_s = small.tile([N, N], F32, tag="sels")
            nc.vector.tensor_scalar(
                sel_s[:], slotv[:], float(s), None, op0=OP.is_equal
            )
            nc.vector.tensor_tensor(sel_s[:], sel_s[:], mask[:], OP.mult)
            nc.vector.tensor_tensor(scr8[:], sel_s[:], wv[:], OP.mult)
            nc.vector.tensor_reduce(wslot[:, s : s + 1], scr8[:], AX.X, OP.add)
            nc.vector.tensor_tensor(scr8[:], sel_s[:], iota8[:], OP.mult)
            nc.vector.tensor_reduce(jslot[:, s : s + 1], scr8[:], AX.X, OP.add)

        # extract this core's two chunk rows via one-hot matmul
        oh = small.tile([P, 2], F32, tag="oh")
        nc.sync.dma_start(oh[:], dd["oh"][:])
        cii = small.tile([1, 2], I32, tag="cii")
        nc.sync.dma_start(cii[:], dd["ci"][:])
        wrow = small.tile([1, 2, 4], F32, tag="wrow")
        jrow_i = small.tile([1, 2, 4], I32, tag="jrowi")
        for li in range(2):
            pr = psumS.tile([1, 4], F32, tag="ps")
            nc.tensor.matmul(
                pr[:], oh[:N, li : li + 1], wslot[:], start=True, stop=True
            )
            nc.vector.tensor_copy(wrow[:, li], pr[:])
            pr2 = psumS.tile([1, 4], F32, tag="ps")
            nc.tensor.matmul(
                pr2[:], oh[:N, li : li + 1], jslot[:], start=True, stop=True
            )
            nc.vector.tensor_copy(jrow_i[:, li], pr2[:])
        wcol = small.tile([P, 2, 4], F32, tag="wcol")
        nc.gpsimd.partition_broadcast(wcol[:], wrow[:])

        # ---- build extended H chunks ----
        for li in range(2):
            eng = nc.vector
            i_reg = nc.values_load(
                cii[0:1, li : li + 1], min_val=0, max_val=N - 1,
                skip_runtime_bounds_check=True,
            )
            for s in range(3):
                j_reg = nc.values_load(
                    jrow_i[0:1, li, s : s + 1], min_val=0, max_val=N - 1,
                    skip_runtime_bounds_check=True,
                )
                eng.tensor_scalar_mul(
                    Hs[li][:, :, ds(s * C, C)],
                    xe[:, :, ds(j_reg * C, C)],
                    wcol[:, li, s : s + 1],
                )
            eng.tensor_copy(
                Hs[li][:, :, ds(3 * C, C)], xe[:, :, ds(i_reg * C, C)]
            )


def _layers(nc, tc, Hs, ident_f, ident_b, ones_b, ones_row, dd):
    with (
        tc.tile_pool(name="statB", bufs=1) as statp,
        tc.tile_pool(name="xnB", bufs=2) as xnp,
        tc.tile_pool(name="xpB", bufs=1) as xpp,
        tc.tile_pool(name="btB", bufs=1) as btp,
        tc.tile_pool(name="workB", bufs=3) as work,
        tc.tile_pool(name="smallB", bufs=2) as small,
        tc.tile_pool(name="w1p", bufs=2) as w1p,
        tc.tile_pool(name="wlp", bufs=1) as wlp,
        tc.tile_pool(name="mtp", bufs=2) as mtp,
        tc.tile_pool(name="psumB", bufs=5, space="PSUM") as psum,
        tc.tile_pool(name="psumT", bufs=2, space="PSUM") as psumT,
        tc.tile_pool(name="psumX", bufs=1, space="PSUM") as psumS,
    ):
        def emit_stats(cs):
            # rmsnorm stats + xn for chunk-step cs; emitted one step early so
            # the (strict-FIFO) engine queues can run it under prior PE work
            hc = Hs[cs % 2]
            inv_b = statp.tile([P, TL], F32, tag="invb", name="invb")
            for nh in range(2):
                sq = xnp.tile([P, EK, 512], BF16, tag="xn", name="sq")
                nc.vector.tensor_tensor(
                    sq[:], hc[:, :, ds(nh * 512, 512)],
                    hc[:, :, ds(nh * 512, 512)], OP.mult,
                )
                pb = psum.tile([P, 512], F32, tag="mm", name="pb")
                for k in range(EK):
                    nc.tensor.matmul(
                        pb[:], ones_b[:], sq[:, k],
                        start=(k == 0), stop=(k == EK - 1),
                    )
                nc.scalar.activation(
                    inv_b[:, ds(nh * 512, 512)], pb[:], AF.Sqrt,
                    bias=dd["eps10"][:], scale=1.0 / float(E),
                )
                nc.vector.reciprocal_approx_fast(
                    inv_b[:, ds(nh * 512, 512)], inv_b[:, ds(nh * 512, 512)]
                )
            xn = xnp.tile([P, EK, TL], BF16, tag="xn", name="xn")
            for k in range(EK):
                nc.vector.tensor_tensor(xn[:, k], hc[:, k], inv_b[:], OP.mult)
            return xn

        xn_cur = None
        fwt = ebt = fbt = None
        for cs in range(2 * L):
            l, li = cs // 2, cs % 2
            if li == 0:
                fwt = wlp.tile([P, PK, E], BF16, tag="fwt", name="fwt")
                nc.sync.dma_start(fwt[:], dd["fwt"][l])
                ebt = small.tile([P, EDK], F32, tag="ebt", name="ebt")
                nc.sync.dma_start(ebt[:], dd["eb"][l])
                fbt = small.tile([P, EK], F32, tag="fbt", name="fbt")
                nc.sync.dma_start(fbt[:], dd["fb"][l])
            hc = Hs[li]
            if cs == 0:
                xn_cur = emit_stats(0)
            xn = xn_cur

            # --- enricher: xp = relu(xn @ W1'^T + eb)^2, feature-major ---
            xp_a = xpp.tile([P, EK, TL], BF16, tag="xp_a", name="xp_a")
            xp_b = xpp.tile([P, EK, TL], BF16, tag="xp_b", name="xp_b")
            xp_x1 = xpp.tile([P, 2 * EK, TL], BF16, tag="xp_x1", name="xp_x1")
            for mg in range(EDK // 4):  # stream W1'^T in 512-col groups
                w1s = w1p.tile([P, EK, 512], BF16, tag="w1s", name="w1s")
                nc.sync.dma_start(w1s[:], dd["w1t"][l][:, :, ds(mg * 512, 512)])
                for ml in range(4):
                    m = mg * 4 + ml
                    if m < EK:
                        dstt, dm = xp_a, m
                    elif m < 2 * EK:
                        dstt, dm = xp_b, m - EK
                    else:
                        dstt, dm = xp_x1, m - 2 * EK
                    pes = [psum.tile([P, 512], F32, tag="mm", name=f"pe{x}") for x in range(2)]
                    for k in range(EK):
                        for nh in range(2):
                            nc.tensor.matmul(
                                pes[nh][:], w1s[:, k, ts(ml, P)],
                                xn[:, k, ds(nh * 512, 512)],
                                start=(k == 0), stop=(k == EK - 1),
                            )
                    for nh in range(2):
                        rel = work.tile([P, 512], BF16, tag="rel", name="rel")
                        nc.scalar.activation(
                            rel[:], pes[nh][:], AF.Relu, bias=ebt[:, m : m + 1]
                        )
                        nc.vector.tensor_tensor(
                            dstt[:, dm, ds(nh * 512, 512)], rel[:], rel[:],
                            OP.mult,
                        )

            # --- a token-major via PE transpose (batched per t-tile) ---
            a_tok = xnp.tile([P, TLK, E], BF16, tag="atok", bufs=1, name="a_tok")
            for tt in range(TLK):
                ptb = psumT.tile([P, E], BF16, tag="ptb", name="ptb")
                for f in range(EK):
                    nc.tensor.transpose(
                        ptb[:, ts(f, P)], xp_a[:, f, ts(tt, P)], ident_b[:]
                    )
                nc.vector.tensor_copy(a_tok[:, tt], ptb[:])

            # --- G = a a^T -> Bt = mask * G * inv_a[cand]  (triangular) ---
            # compact Bt: mi<4 -> slots 2mi (cols 0:512), 2mi+1 (512:);
            #             mi>=4 -> slot 4+mi (cols 512:)
            Bt = btp.tile([P, 12, 512], BF16, tag="bt", name="Bt")
            inv8 = small.tile([P, N], F32, tag="inv8", name="inv8")
            invr = small.tile([1, TL], F32, tag="invr", bufs=1, name="invr")
            for mi in range(TLK):
                mtt = mtp.tile([P, TL], BF16, tag="mtt", name="mtt")
                nc.sync.dma_start(mtt[:], dd["mt"][l, mi])
                blocks = (0, 1) if mi < 4 else (1,)
                pgs = {b: psum.tile([P, 512], F32, tag="mm", name=f"pg{b}") for b in blocks}
                for k in range(EK):
                    for b in blocks:
                        nc.tensor.matmul(
                            pgs[b][:], xp_a[:, k, ts(mi, P)],
                            xp_a[:, k, ds(b * 512, 512)],
                            start=(k == 0), stop=(k == EK - 1),
                        )
                # diagonal -> inv_a for this tile's tokens
                bd = mi // 4
                off = (mi % 4) * P
                dscr = work.tile([P, P], F32, tag="dscr", name="dscr")
                nc.vector.tensor_tensor(
                    dscr[:], pgs[bd][:, ds(off, P)], ident_f[:], OP.mult
                )
                nc.vector.tensor_reduce(
                    inv8[:, mi : mi + 1], dscr[:], AX.X, OP.add
                )
                nc.scalar.activation(
                    inv8[:, mi : mi + 1], inv8[:, mi : mi + 1], AF.Sqrt,
                    bias=dd["eps8"][:],
                )
                nc.vector.reciprocal_approx_fast(
                    inv8[:, mi : mi + 1], inv8[:, mi : mi + 1]
                )
                # Bt = (pg * inv_a[cand]) * mask   (one fused DVE op each)
                for b in blocks:
                    slot = 2 * mi + b if mi < 4 else 4 + mi
                    nc.vector.scalar_tensor_tensor(
                        Bt[:, slot], pgs[b][:],
                        inv8[:, mi : mi + 1], mtt[:, ds(b * 512, 512)],
                        op0=OP.mult, op1=OP.mult,
                    )
                # row for the query-side broadcast
                pr = psumS.tile([1, P], F32, tag="px", name="pr")
                nc.tensor.transpose(pr[:], inv8[:, mi : mi + 1], ident_f[:])
                nc.vector.tensor_copy(invr[:, ts(mi, P)], pr[:])

            # inv_a broadcast rows [P, TL] via K=1 matmul
            inv_cb = statp.tile([P, TL], F32, tag="invcb", bufs=1, name="inv_cb")
            for nh in range(2):
                pbc = psumS.tile([P, 512], F32, tag="px", name="pbc")
                nc.tensor.matmul(
                    pbc[:], ones_row[:, :P], invr[:, ds(nh * 512, 512)],
                    start=True, stop=True,
                )
                nc.vector.tensor_copy(inv_cb[:, ds(nh * 512, 512)], pbc[:])

            # --- attn + ab (ab overwrites xp_a in place; triangular) ---
            # xp_b * inv_cb fold interleaved per-f to avoid a DVE hump
            for f in range(EK):
                nc.vector.tensor_tensor(
                    xp_b[:, f], xp_b[:, f], inv_cb[:], OP.mult
                )
                pa1 = psum.tile([P, 512], F32, tag="mm", name="pa1")
                pa0 = psum.tile([P, 512], F32, tag="mm", name="pa0")
                for kc in range(TLK):
                    s1 = 2 * kc + 1 if kc < 4 else 4 + kc
                    nc.tensor.matmul(
                        pa1[:], a_tok[:, kc, ts(f, P)],
                        Bt[:, s1],
                        start=(kc == 0), stop=(kc == TLK - 1),
                    )
                    if kc < 4:
                        nc.tensor.matmul(
                            pa0[:], a_tok[:, kc, ts(f, P)],
                            Bt[:, 2 * kc],
                            start=(kc == 0), stop=(kc == 3),
                        )
                for nh, pa in ((0, pa0), (1, pa1)):
                    nc.vector.tensor_tensor(
                        xp_a[:, f, ds(nh * 512, 512)], pa[:],
                        xp_b[:, f, ds(nh * 512, 512)], OP.mult,
                    )

            # stats for the NEXT chunk-step, ahead of this fuser, so the
            # FIFO engine queues drain them under the fuser MM stream
            xn_next = emit_stats(cs + 1) if cs + 1 < 2 * L else None

            # --- fuser: H += cat @ fw'^T + fb  (bias+residual fused) ---
            for m in range(EK):
                pfs = [psum.tile([P, 512], F32, tag="mm", name=f"pf{x}") for x in range(2)]
                for kp in range(PK):
                    rhs = (
                        xp_a[:, kp]
                        if kp < EK
                        else xp_x1[:, kp - EK]
                    )
                    for nh in range(2):
                        nc.tensor.matmul(
                            pfs[nh][:], fwt[:, kp, ts(m, P)],
                            rhs[:, ds(nh * 512, 512)],
                            start=(kp == 0), stop=(kp == PK - 1),
                        )
                for nh in range(2):
                    nc.vector.scalar_tensor_tensor(
                        hc[:, m, ds(nh * 512, 512)], pfs[nh][:],
                        fbt[:, m : m + 1], hc[:, m, ds(nh * 512, 512)],
                        op0=OP.add, op1=OP.add,
                    )
            xn_cur = xn_next


def _logits(nc, tc, Hs, ones_b, dd, out_d):
    with (
        tc.tile_pool(name="bigC", bufs=1) as big,
        tc.tile_pool(name="workC", bufs=3) as work,
        tc.tile_pool(name="wvp", bufs=2) as wvp,
        tc.tile_pool(name="psumC", bufs=7, space="PSUM") as psum,
        tc.tile_pool(name="psumCF", bufs=1, space="PSUM") as psumF,
    ):
        fin_bf = big.tile([P, EK, 512], BF16, tag="fin")
        sqf = big.tile([P, EK, 512], BF16, tag="sqf")
        inv_f = big.tile([P, 512], F32, tag="invf")
        for li in range(2):
            hs = Hs[li][:, :, ds(3 * C, C)]
            nc.vector.tensor_tensor(sqf[:, :, ds(li * C, C)], hs, hs, OP.mult)
            pbf = psumF.tile([P, 256], F32, tag="mmf")
            for k in range(EK):
                nc.tensor.matmul(
                    pbf[:], ones_b[:], sqf[:, k, ds(li * C, C)],
                    start=(k == 0), stop=(k == EK - 1),
                )
            nc.scalar.activation(
                inv_f[:, ds(li * C, C)], pbf[:], AF.Sqrt,
                bias=dd["eps10"][:], scale=1.0 / float(E),
            )
            nc.vector.reciprocal_approx_fast(
                inv_f[:, ds(li * C, C)], inv_f[:, ds(li * C, C)]
            )
            for k in range(EK):
                nc.vector.tensor_tensor(
                    fin_bf[:, k, ds(li * C, C)],
                    Hs[li][:, k, ds(3 * C, C)],
                    inv_f[:, ds(li * C, C)], OP.mult,
                )

        NB = 512
        n_blocks = (V + NB - 1) // NB  # 63
        for vb in range(n_blocks):
            b0 = vb * NB
            bw = min(NB, V - b0)
            wv_t = wvp.tile([P, EK, NB], BF16, tag=f"wv{vb % 3}", bufs=2,
                            name=f"wv{vb % 3}")
            nc.scalar.dma_start(wv_t[:, :, :bw], dd["wtet"][:, :, ds(b0, bw)])
            for m in range(4):
                pl = psum.tile([P, 512], F32, tag="mm", name="pl")
                for k in range(EK):
                    nc.tensor.matmul(
                        pl[:, :bw], fin_bf[:, k, ts(m, P)],
                        wv_t[:, k, :bw],
                        start=(k == 0), stop=(k == EK - 1),
                    )
                ot = work.tile([P, 512], BF16, tag="ot")
                nc.vector.tensor_copy(ot[:, :bw], pl[:, :bw])
                nc.sync.dma_start(
                    out_d[:, m, ds(b0, bw)], ot[:, :bw]
                )


def _emit(nc):
    dd = {
        "xe": nc.dram_tensor("xe", (P, EK, T), F32, kind="ExternalInput"),
        "xnb": nc.dram_tensor("xnb", (P, EK, T), BF16, kind="ExternalInput"),
        "w1t": nc.dram_tensor("w1t", (L, P, EK, ED), BF16, kind="ExternalInput"),
        "eb": nc.dram_tensor("eb", (L, P, EDK), F32, kind="ExternalInput"),
        "fwt": nc.dram_tensor("fwt", (L, P, PK, E), BF16, kind="ExternalInput"),
        "fb": nc.dram_tensor("fb", (L, P, EK), F32, kind="ExternalInput"),
        "mt": nc.dram_tensor("mt", (L, TLK, P, TL), BF16, kind="ExternalInput"),
        "wtet": nc.dram_tensor("wtet", (P, EK, V), BF16, kind="ExternalInput"),
        "oh": nc.dram_tensor("oh", (P, 2), F32, kind="ExternalInput"),
        "ci": nc.dram_tensor("ci", (1, 2), I32, kind="ExternalInput"),
        "nselcol": nc.dram_tensor("nselcol", (N, N), F32, kind="ExternalInput"),
        "nselrow": nc.dram_tensor("nselrow", (N, 1), F32, kind="ExternalInput"),
        "iota8": nc.dram_tensor("iota8", (N, N), F32, kind="ExternalInput"),
    }
    out_d = nc.dram_tensor("out", (P, 2 * C // P, V), BF16, kind="ExternalOutput")

    with tile.TileContext(nc) as tc:
        with tc.tile_pool(name="persist", bufs=1) as persist:
            ident_f = persist.tile([P, P], F32)
            make_identity(nc, ident_f[:])
            ident_b = persist.tile([P, P], BF16)
            make_identity(nc, ident_b[:])
            ones_b = persist.tile([P, P], BF16)
            nc.vector.memset(ones_b[:], 1.0)
            ones_row = persist.tile([1, P], F32)
            nc.vector.memset(ones_row[:], 1.0)
            ones_col = persist.tile([P, 1], F32)
            nc.vector.memset(ones_col[:], 1.0)
            dd["ones_col"] = ones_col
            eps10 = persist.tile([P, 1], F32)
            nc.vector.memset(eps10[:], 1.0e-10)
            eps8 = persist.tile([P, 1], F32)
            nc.vector.memset(eps8[:], 1.0e-8)
            dd["eps10"] = eps10
            dd["eps8"] = eps8
            H0 = persist.tile([P, EK, TL], F32)
            H1 = persist.tile([P, EK, TL], F32)
            Hs = (H0, H1)

            _phase_a(nc, tc, Hs, ident_f, ones_row, dd)
            _layers(nc, tc, Hs, ident_f, ident_b, ones_b, ones_row, dd)
            _logits(nc, tc, Hs, ones_b, dd, out_d)

    return nc


_CACHE = {}


def _get_compiled():
    if "nc" not in _CACHE:
        if LDWOPT:
            from concourse.compiler_utils import (
                get_compiler_flags,
                set_compiler_flags,
            )

            flags = get_compiler_flags()
            new_flags = []
            for f in flags:
                if f.startswith("--internal-backend-options="):
                    f = f.replace("--enable-ldw-opt=false", "--enable-ldw-opt=true")
                new_flags.append(f)
            set_compiler_flags(new_flags)
        nc = bacc.Bacc("TRN2", debug=False, num_devices=8)
        _emit(nc)
        nc.compile()
        _CACHE["nc"] = nc
    return _CACHE["nc"]


def _prep_host(inputs):
    wte = np.asarray(inputs["wte"], np.float32)
    rms_w = np.asarray(inputs["rms_w"], np.float32)
    enr_w = np.asarray(inputs["enr_w"], np.float32)
    enr_b = np.asarray(inputs["enr_b"], np.float32)
    spatial = np.asarray(inputs["spatial"], np.float32)
    fus_w = np.asarray(inputs["fus_w"], np.float32)
    fus_b = np.asarray(inputs["fus_b"], np.float32)
    lnf_w = np.asarray(inputs["lnf_w"], np.float32)

    bf = ml_dtypes.bfloat16
    w1 = enr_w * rms_w[:, None, :]  # fold rms weight
    w1t = np.ascontiguousarray(
        w1.transpose(0, 2, 1).reshape(L, EK, P, ED).transpose(0, 2, 1, 3)
    ).astype(bf)
    eb = np.ascontiguousarray(
        enr_b.reshape(L, EDK, P).transpose(0, 2, 1)
    ).astype(np.float32)
    fwt = np.ascontiguousarray(
        fus_w.transpose(0, 2, 1).reshape(L, PK, P, E).transpose(0, 2, 1, 3)
    ).astype(bf)
    fb = np.ascontiguousarray(
        fus_b.reshape(L, EK, P).transpose(0, 2, 1)
    ).astype(np.float32)
    mt = np.stack([np.tril(spatial[l]).T for l in range(L)])
    mt = np.ascontiguousarray(mt.reshape(L, TLK, P, TL)).astype(bf)
    wtet = np.ascontiguousarray(
        (wte * lnf_w[None, :]).T.reshape(EK, P, V).transpose(1, 0, 2)
    ).astype(bf)

    nselcol = np.zeros((N, N), np.float32)
    nselrow = np.zeros((N, 1), np.float32)
    for i in range(N):
        n_sel = min(i, 3)
        if n_sel > 0:
            nselcol[i, n_sel - 1] = 1.0
        nselrow[i, 0] = float(2 - n_sel)
    iota8 = np.broadcast_to(np.arange(N, dtype=np.float32)[None, :], (N, N)).copy()

    # embedding gather + E-major layout per batch (+ cosine-normalized bf16)
    ids = np.asarray(inputs["input_ids"], np.int32)
    xes, xnbs = [], []
    for b in range(B):
        x = wte[ids[b]]  # [T, E]
        xe = np.ascontiguousarray(x.T.reshape(EK, P, T).transpose(1, 0, 2))
        xn = x / (np.linalg.norm(x, axis=-1, keepdims=True) + 1e-8)
        xnb = np.ascontiguousarray(
            xn.T.reshape(EK, P, T).transpose(1, 0, 2)
        ).astype(bf)
        xes.append(xe.astype(np.float32))
        xnbs.append(xnb)

    return dict(
        w1t=w1t, eb=eb, fwt=fwt, fb=fb, mt=mt, wtet=wtet,
        nselcol=nselcol, nselrow=nselrow, iota8=iota8,
        _xes=xes, _xnbs=xnbs,
    )


def _make_in_maps(inputs, shared):
    sh = {k: v for k, v in shared.items() if not k.startswith("_")}
    in_maps = []
    for c in range(8):
        b = c // 4
        i0 = 2 * (c % 4)
        oh = np.zeros((P, 2), np.float32)
        oh[i0, 0] = 1.0
        oh[i0 + 1, 1] = 1.0
        ci = np.array([[i0, i0 + 1]], np.int32)
        in_maps.append({
            **sh, "xe": shared["_xes"][b], "xnb": shared["_xnbs"][b],
            "oh": oh, "ci": ci,
        })
    return in_maps


def kernel(**inputs):
    shared = _prep_host(inputs)
    nc = _get_compiled()
    in_maps = _make_in_maps(inputs, shared)

    res = run_bass_kernel_spmd(nc, in_maps, core_ids=list(range(8)))
    outs = [r["out"] for r in res.results]  # each [P, 4, V] bf16
    full = np.stack(
        [o.astype(np.float32).transpose(1, 0, 2).reshape(2 * C, V) for o in outs]
    ).reshape(B, 4, 2 * C, V).reshape(B, T, V)
    return full


# revision 26
# speedup vs baseline: 1.0426x; 1.0426x over previous
"""Trainium2 Bass kernel for nn_Avey (retrieval-knn block transformer).

Sharding: 8 cores; core c handles batch b=c//4, chunks i0=2*(c%4), i0+1.
Each core is fully independent (no collectives):
  - host ships E-major embeddings (f32) + cosine-normalized bf16 copy
  - retrieval scores for all (i,j) chunk pairs of its batch (bf16 GEMMs,
    free-dim max; replicated across the 4 cores of a batch so the SPMD
    program is uniform)
  - top-k selection via vector ops, weighted chunk-select via dynamic slices
  - 4 block layers (bf16 GEMMs, fp32 residual/stats, triangular masked
    cosine-sim attention)
  - logits GEMM over the full vocab for its 512 output tokens (bf16 out)
Host side does layout prep of constant weights (transpose/cast/fold) and
the embedding gather/normalize.
"""
import sys
import os

sys.path.insert(0, "/opt/trn_rl_repo")

import numpy as np
import ml_dtypes

import concourse.bass as bass
import concourse.bacc as bacc
import concourse.mybir as mybir
import concourse.tile as tile
from concourse.bass import ds, ts
from concourse.bass_utils import run_bass_kernel_spmd
from concourse.masks import make_identity

P = 128
V, E, L = 32000, 768, 4
C, TL = 256, 1024
ED = 3072
PROJ_IN = 2304
B, T = 2, 2048
N = T // C  # 8 chunks per batch
EK = E // P  # 6
EDK = ED // P  # 24
PK = PROJ_IN // P  # 18
TLK = TL // P  # 8
F32 = mybir.dt.float32
BF16 = mybir.dt.bfloat16
I32 = mybir.dt.int32
AF = mybir.ActivationFunctionType
OP = mybir.AluOpType
AX = mybir.AxisListType

NEG = -1.0e30
LDWOPT = bool(int(os.environ.get("AVEY_LDWOPT", "0")))


def _phase_a(nc, tc, Hs, ident_f, ones_row, dd):
    """Scores + selection + extended-H build (from host-shipped embeddings)."""
    with (
        tc.tile_pool(name="bigA", bufs=1) as bigA,
        tc.tile_pool(name="workA", bufs=2) as work,
        tc.tile_pool(name="smallA", bufs=2) as small,
        tc.tile_pool(name="psumA", bufs=5, space="PSUM") as psum,
        tc.tile_pool(name="psumAS", bufs=1, space="PSUM") as psumS,
        tc.tile_pool(name="psumACC", bufs=1, space="PSUM") as psumACC,
    ):
        xnb = bigA.tile([P, EK, T], BF16, tag="xnb")
        for g in range(4):  # split so early chunks land first
            nc.sync.dma_start(
                xnb[:, :, ds(g * 512, 512)], dd["xnb"][:, :, ds(g * 512, 512)]
            )
        xe = bigA.tile([P, EK, T], F32, tag="xe")
        nc.sync.dma_start(xe[:], dd["xe"][:])

        # sel8[:, i] = ones in column i (same on every partition)
        sel8 = small.tile([P, N, N], F32, tag="sel8")
        nc.vector.memset(sel8[:], 0.0)
        for i in range(N):
            nc.vector.memset(sel8[:, i, i : i + 1], 1.0)

        # ---- scores: for each chunk i, cos-sims vs all candidate cols ----
        # row i of pss accumulates scores for chunk i via one-hot lhsT;
        # the pss MMs are deferred so the PE FIFO never waits on reduces
        pss = psumACC.tile([N, N], F32, tag="pss")
        rmts = []
        for i in range(1, N):
            nb = (i * C + 511) // 512  # 512-wide candidate blocks
            rmt = work.tile([P, 2, N], F32, tag=f"rmt{i}", name=f"rmt{i}")
            nc.vector.memset(rmt[:], NEG / 512.0)
            rmts.append(rmt)
            for ci in range(2):
                pgs = []
                for b in range(nb):
                    cw = min(512, i * C - b * 512)
                    pg = psum.tile([P, 512], F32, tag="mm", name=f"pg{b}")
                    pgs.append((pg, cw))
                for k in range(EK):
                    for b, (pg, cw) in enumerate(pgs):
                        nc.tensor.matmul(
                            pg[:, :cw],
                            xnb[:, k, ds(i * C + ci * P, P)],
                            xnb[:, k, ds(b * 512, cw)],
                            start=(k == 0),
                            stop=(k == EK - 1),
                        )
                for b, (pg, cw) in enumerate(pgs):
                    nj = cw // C  # 1 or 2 chunks in this block
                    nc.vector.tensor_reduce(
                        rmt[:, ci, ds(2 * b, nj)],
                        pg[:, :cw].rearrange("p (j c) -> p j c", c=C),
                        AX.X,
                        OP.max,
                    )
        for i in range(1, N):
            for ci in range(2):
                nc.tensor.matmul(
                    pss[:],
                    sel8[:, i],
                    rmts[i - 1][:, ci],
                    start=(i == 1 and ci == 0),
                    stop=(i == N - 1 and ci == 1),
                )

        # ---- selection math on [N, N] rows ----
        srows16 = small.tile([N, 2 * N], F32, tag="srows")
        nc.vector.memset(srows16[:], NEG)
        nc.vector.tensor_copy(srows16[:, :N], pss[:])
        srows = srows16[:, :N]
        nselcol = small.tile([N, N], F32, tag="nselcol")
        nselrow = small.tile([N, 1], F32, tag="nselrow")
        iota8 = small.tile([N, N], F32, tag="iota8")
        nc.sync.dma_start(nselcol[:], dd["nselcol"][:])
        nc.sync.dma_start(nselrow[:], dd["nselrow"][:])
        nc.sync.dma_start(iota8[:], dd["iota8"][:])

        maxv = small.tile([N, 8], F32, tag="maxv")
        nc.vector.max(maxv[:], srows16[:])
        kth = small.tile([N, 1], F32, tag="kth")
        scr8 = small.tile([N, N], F32, tag="scr8")
        nc.vector.tensor_tensor(scr8[:], maxv[:], nselcol[:], OP.mult)
        nc.vector.tensor_reduce(kth[:], scr8[:], AX.X, OP.add)
        mask = small.tile([N, N], F32, tag="mask")
        nc.vector.tensor_scalar(mask[:], srows, kth[:], None, op0=OP.is_ge)
        # cumsum over 8 via 3 shift-adds (ping-pong)
        cumA = small.tile([N, N], F32, tag="cumA")
        cumB = small.tile([N, N], F32, tag="cumB")
        nc.vector.tensor_copy(cumA[:], mask[:])
        pairs = ((cumA, cumB), (cumB, cumA), (cumA, cumB))
        for sh, (src, dst) in zip((1, 2, 4), pairs):
            nc.vector.tensor_copy(dst[:, :sh], src[:, :sh])
            nc.vector.tensor_tensor(dst[:, sh:], src[:, sh:], src[:, : N - sh], OP.add)
        cum = cumB
        # first selected: fs = mask * (cum == 1); w = srows / (s_first + 1e-8)
        fs = small.tile([N, N], F32, tag="fs")
        nc.vector.tensor_scalar(fs[:], cum[:], 1.0, None, op0=OP.is_equal)
        nc.vector.tensor_tensor(fs[:], fs[:], mask[:], OP.mult)
        s_first = small.tile([N, 1], F32, tag="sfirst")
        nc.vector.tensor_tensor(scr8[:], fs[:], srows, OP.mult)
        nc.vector.tensor_reduce(s_first[:], scr8[:], AX.X, OP.add)
        nc.vector.tensor_scalar_add(s_first[:], s_first[:], 1.0e-8)
        nc.vector.reciprocal(s_first[:], s_first[:])
        wv = small.tile([N, N], F32, tag="wv")
        nc.vector.tensor_scalar_mul(wv[:], srows, s_first[:])
        # slotv = cum + (2 - n_sel)
        slotv = small.tile([N, N], F32, tag="slotv")
        nc.vector.tensor_scalar(slotv[:], cum[:], nselrow[:], None, op0=OP.add)
        # per-slot weight / source index  [N, 4]
        wslot = small.tile([N, 4], F32, tag="wslot")
        jslot = small.tile([N, 4], F32, tag="jslot")
        nc.vector.memset(wslot[:], 0.0)
        nc.vector.memset(jslot[:], 0.0)
        for s in range(3):
            sel_s = small.tile([N, N], F32, tag="sels")
            nc.vector.tensor_scalar(
                sel_s[:], slotv[:], float(s), None, op0=OP.is_equal
            )
            nc.vector.tensor_tensor(sel_s[:], sel_s[:], mask[:], OP.mult)
            nc.vector.tensor_tensor(scr8[:], sel_s[:], wv[:], OP.mult)
            nc.vector.tensor_reduce(wslot[:, s : s + 1], scr8[:], AX.X, OP.add)
            nc.vector.tensor_tensor(scr8[:], sel_s[:], iota8[:], OP.mult)
            nc.vector.tensor_reduce(jslot[:, s : s + 1], scr8[:], AX.X, OP.add)

        # extract this core's two chunk rows via one-hot matmul
        oh = small.tile([P, 2], F32, tag="oh")
        nc.sync.dma_start(oh[:], dd["oh"][:])
        cii = small.tile([1, 2], I32, tag="cii")
        nc.sync.dma_start(cii[:], dd["ci"][:])
        wrow = small.tile([1, 2, 4], F32, tag="wrow")
        jrow_i = small.tile([1, 2, 4], I32, tag="jrowi")
        for li in range(2):
            pr = psumS.tile([1, 4], F32, tag="ps")
            nc.tensor.matmul(
                pr[:], oh[:N, li : li + 1], wslot[:], start=True, stop=True
            )
            nc.vector.tensor_copy(wrow[:, li], pr[:])
            pr2 = psumS.tile([1, 4], F32, tag="ps")
            nc.tensor.matmul(
                pr2[:], oh[:N, li : li + 1], jslot[:], start=True, stop=True
            )
            nc.vector.tensor_copy(jrow_i[:, li], pr2[:])
        wcol = small.tile([P, 2, 4], F32, tag="wcol")
        nc.gpsimd.partition_broadcast(wcol[:], wrow[:])

        # ---- build extended H chunks ----
        for li in range(2):
            eng = nc.vector
            i_reg = nc.values_load(
                cii[0:1, li : li + 1], min_val=0, max_val=N - 1,
                skip_runtime_bounds_check=True,
            )
            for s in range(3):
                j_reg = nc.values_load(
                    jrow_i[0:1, li, s : s + 1], min_val=0, max_val=N - 1,
                    skip_runtime_bounds_check=True,
                )
                eng.tensor_scalar_mul(
                    Hs[li][:, :, ds(s * C, C)],
                    xe[:, :, ds(j_reg * C, C)],
                    wcol[:, li, s : s + 1],
                )
            eng.tensor_copy(
                Hs[li][:, :, ds(3 * C, C)], xe[:, :, ds(i_reg * C, C)]
            )


def _layers(nc, tc, Hs, ident_f, ident_b, ones_b, ones_row, dd):
    with (
        tc.tile_pool(name="statB", bufs=1) as statp,
        tc.tile_pool(name="xnB", bufs=2) as xnp,
        tc.tile_pool(name="xpB", bufs=1) as xpp,
        tc.tile_pool(name="btB", bufs=1) as btp,
        tc.tile_pool(name="workB", bufs=3) as work,
        tc.tile_pool(name="smallB", bufs=2) as small,
        tc.tile_pool(name="w1p", bufs=2) as w1p,
        tc.tile_pool(name="wlp", bufs=1) as wlp,
        tc.tile_pool(name="mtp", bufs=2) as mtp,
        tc.tile_pool(name="psumB", bufs=5, space="PSUM") as psum,
        tc.tile_pool(name="psumT", bufs=2, space="PSUM") as psumT,
        tc.tile_pool(name="psumX", bufs=1, space="PSUM") as psumS,
    ):
        def emit_stats(cs):
            # rmsnorm stats + xn for chunk-step cs; emitted one step early so
            # the (strict-FIFO) engine queues can run it under prior PE work
            hc = Hs[cs % 2]
            inv_b = statp.tile([P, TL], F32, tag="invb", name="invb")
            for nh in range(2):
                sq = xnp.tile([P, EK, 512], BF16, tag="xn", name="sq")
                nc.vector.tensor_tensor(
                    sq[:], hc[:, :, ds(nh * 512, 512)],
                    hc[:, :, ds(nh * 512, 512)], OP.mult,
                )
                pb = psum.tile([P, 512], F32, tag="mm", name="pb")
                for k in range(EK):
                    nc.tensor.matmul(
                        pb[:], ones_b[:], sq[:, k],
                        start=(k == 0), stop=(k == EK - 1),
                    )
                nc.scalar.activation(
                    inv_b[:, ds(nh * 512, 512)], pb[:], AF.Sqrt,
                    bias=dd["eps10"][:], scale=1.0 / float(E),
                )
                nc.vector.reciprocal_approx_fast(
                    inv_b[:, ds(nh * 512, 512)], inv_b[:, ds(nh * 512, 512)]
                )
            xn = xnp.tile([P, EK, TL], BF16, tag="xn", name="xn")
            for k in range(EK):
                nc.vector.tensor_tensor(xn[:, k], hc[:, k], inv_b[:], OP.mult)
            return xn

        xn_cur = None
        fwt = ebt = fbt = None
        for cs in range(2 * L):
            l, li = cs // 2, cs % 2
            if li == 0:
                fwt = wlp.tile([P, PK, E], BF16, tag="fwt", name="fwt")
                nc.sync.dma_start(fwt[:], dd["fwt"][l])
                ebt = small.tile([P, EDK], F32, tag="ebt", name="ebt")
                nc.sync.dma_start(ebt[:], dd["eb"][l])
                fbt = small.tile([P, EK], F32, tag="fbt", name="fbt")
                nc.sync.dma_start(fbt[:], dd["fb"][l])
            hc = Hs[li]
            if cs == 0:
                xn_cur = emit_stats(0)
            xn = xn_cur

            # --- enricher: xp = relu(xn @ W1'^T + eb)^2, feature-major ---
            xp_a = xpp.tile([P, EK, TL], BF16, tag="xp_a", name="xp_a")
            xp_b = xpp.tile([P, EK, TL], BF16, tag="xp_b", name="xp_b")
            xp_x1 = xpp.tile([P, 2 * EK, TL], BF16, tag="xp_x1", name="xp_x1")
            for mg in range(EDK // 4):  # stream W1'^T in 512-col groups
                w1s = w1p.tile([P, EK, 512], BF16, tag="w1s", name="w1s")
                nc.sync.dma_start(w1s[:], dd["w1t"][l][:, :, ds(mg * 512, 512)])
                for ml in range(4):
                    m = mg * 4 + ml
                    if m < EK:
                        dstt, dm = xp_a, m
                    elif m < 2 * EK:
                        dstt, dm = xp_b, m - EK
                    else:
                        dstt, dm = xp_x1, m - 2 * EK
                    pes = [psum.tile([P, 512], F32, tag="mm", name=f"pe{x}") for x in range(2)]
                    for k in range(EK):
                        for nh in range(2):
                            nc.tensor.matmul(
                                pes[nh][:], w1s[:, k, ts(ml, P)],
                                xn[:, k, ds(nh * 512, 512)],
                                start=(k == 0), stop=(k == EK - 1),
                            )
                    for nh in range(2):
                        rel = work.tile([P, 512], BF16, tag="rel", name="rel")
                        nc.scalar.activation(
                            rel[:], pes[nh][:], AF.Relu, bias=ebt[:, m : m + 1]
                        )
                        nc.vector.tensor_tensor(
                            dstt[:, dm, ds(nh * 512, 512)], rel[:], rel[:],
                            OP.mult,
                        )

            # --- a token-major via PE transpose (batched per t-tile) ---
            a_tok = xnp.tile([P, TLK, E], BF16, tag="atok", bufs=1, name="a_tok")
            for tt in range(TLK):
                ptb = psumT.tile([P, E], BF16, tag="ptb", name="ptb")
                for f in range(EK):
                    nc.tensor.transpose(
                        ptb[:, ts(f, P)], xp_a[:, f, ts(tt, P)], ident_b[:]
                    )
                nc.vector.tensor_copy(a_tok[:, tt], ptb[:])

            # --- G = a a^T -> Bt = mask * G * inv_a[cand]  (triangular) ---
            # compact Bt: mi<4 -> slots 2mi (cols 0:512), 2mi+1 (512:);
            #             mi>=4 -> slot 4+mi (cols 512:)
            Bt = btp.tile([P, 12, 512], BF16, tag="bt", name="Bt")
            inv8 = small.tile([P, N], F32, tag="inv8", name="inv8")
            invr = small.tile([1, TL], F32, tag="invr", bufs=1, name="invr")
            for mi in range(TLK):
                mtt = mtp.tile([P, TL], BF16, tag="mtt", name="mtt")
                nc.sync.dma_start(mtt[:], dd["mt"][l, mi])
                blocks = (0, 1) if mi < 4 else (1,)
                pgs = {b: psum.tile([P, 512], F32, tag="mm", name=f"pg{b}") for b in blocks}
                for k in range(EK):
                    for b in blocks:
                        nc.tensor.matmul(
                            pgs[b][:], xp_a[:, k, ts(mi, P)],
                            xp_a[:, k, ds(b * 512, 512)],
                            start=(k == 0), stop=(k == EK - 1),
                        )
                # diagonal -> inv_a for this tile's tokens
                bd = mi // 4
                off = (mi % 4) * P
                dscr = work.tile([P, P], F32, tag="dscr", name="dscr")
                nc.vector.tensor_tensor(
                    dscr[:], pgs[bd][:, ds(off, P)], ident_f[:], OP.mult
                )
                nc.vector.tensor_reduce(
                    inv8[:, mi : mi + 1], dscr[:], AX.X, OP.add
                )
                nc.scalar.activation(
                    inv8[:, mi : mi + 1], inv8[:, mi : mi + 1], AF.Sqrt,
                    bias=dd["eps8"][:],
                )
                nc.vector.reciprocal_approx_fast(
                    inv8[:, mi : mi + 1], inv8[:, mi : mi + 1]
                )
                # Bt = (pg * inv_a[cand]) * mask   (one fused DVE op each)
                for b in blocks:
                    slot = 2 * mi + b if mi < 4 else 4 + mi
                    nc.vector.scalar_tensor_tensor(
                        Bt[:, slot], pgs[b][:],
                        inv8[:, mi : mi + 1], mtt[:, ds(b * 512, 512)],
                        op0=OP.mult, op1=OP.mult,
                    )
                # row for the query-side broadcast
                pr = psumS.tile([1, P], F32, tag="px", name="pr")
                nc.tensor.transpose(pr[:], inv8[:, mi : mi + 1], ident_f[:])
                nc.vector.tensor_copy(invr[:, ts(mi, P)], pr[:])

            # inv_a broadcast rows [P, TL] via K=1 matmul
            inv_cb = statp.tile([P, TL], F32, tag="invcb", bufs=1, name="inv_cb")
            for nh in range(2):
                pbc = psumS.tile([P, 512], F32, tag="px", name="pbc")
                nc.tensor.matmul(
                    pbc[:], ones_row[:, :P], invr[:, ds(nh * 512, 512)],
                    start=True, stop=True,
                )
                nc.vector.tensor_copy(inv_cb[:, ds(nh * 512, 512)], pbc[:])

            # --- attn + ab (ab overwrites xp_a in place; triangular) ---
            # xp_b * inv_cb fold interleaved per-f to avoid a DVE hump
            for f in range(EK):
                nc.vector.tensor_tensor(
                    xp_b[:, f], xp_b[:, f], inv_cb[:], OP.mult
                )
                pa1 = psum.tile([P, 512], F32, tag="mm", name="pa1")
                pa0 = psum.tile([P, 512], F32, tag="mm", name="pa0")
                for kc in range(TLK):
                    s1 = 2 * kc + 1 if kc < 4 else 4 + kc
                    nc.tensor.matmul(
                        pa1[:], a_tok[:, kc, ts(f, P)],
                        Bt[:, s1],
                        start=(kc == 0), stop=(kc == TLK - 1),
                    )
                    if kc < 4:
                        nc.tensor.matmul(
                            pa0[:], a_tok[:, kc, ts(f, P)],
                            Bt[:, 2 * kc],
                            start=(kc == 0), stop=(kc == 3),
                        )
                for nh, pa in ((0, pa0), (1, pa1)):
                    nc.vector.tensor_tensor(
                        xp_a[:, f, ds(nh * 512, 512)], pa[:],
                        xp_b[:, f, ds(nh * 512, 512)], OP.mult,
                    )

            # stats for the NEXT chunk-step, ahead of this fuser, so the
            # FIFO engine queues drain them under the fuser MM stream
            xn_next = emit_stats(cs + 1) if cs + 1 < 2 * L else None

            # --- fuser: H += cat @ fw'^T + fb  (bias+residual fused) ---
            for m in range(EK):
                pfs = [psum.tile([P, 512], F32, tag="mm", name=f"pf{x}") for x in range(2)]
                for kp in range(PK):
                    rhs = (
                        xp_a[:, kp]
                        if kp < EK
                        else xp_x1[:, kp - EK]
                    )
                    for nh in range(2):
                        nc.tensor.matmul(
                            pfs[nh][:], fwt[:, kp, ts(m, P)],
                            rhs[:, ds(nh * 512, 512)],
                            start=(kp == 0), stop=(kp == PK - 1),
                        )
                for nh in range(2):
                    nc.vector.scalar_tensor_tensor(
                        hc[:, m, ds(nh * 512, 512)], pfs[nh][:],
                        fbt[:, m : m + 1], hc[:, m, ds(nh * 512, 512)],
                        op0=OP.add, op1=OP.add,
                    )
            xn_cur = xn_next


def _logits(nc, tc, Hs, ones_b, dd, out_d):
    with (
        tc.tile_pool(name="bigC", bufs=1) as big,
        tc.tile_pool(name="workC", bufs=3) as work,
        tc.tile_pool(name="wvp", bufs=2) as wvp,
        tc.tile_pool(name="psumC", bufs=7, space="PSUM") as psum,
        tc.tile_pool(name="psumCF", bufs=1, space="PSUM") as psumF,
    ):
        fin_bf = big.tile([P, EK, 512], BF16, tag="fin")
        sqf = big.tile([P, EK, 512], BF16, tag="sqf")
        inv_f = big.tile([P, 512], F32, tag="invf")
        for li in range(2):
            hs = Hs[li][:, :, ds(3 * C, C)]
            nc.vector.tensor_tensor(sqf[:, :, ds(li * C, C)], hs, hs, OP.mult)
            pbf = psumF.tile([P, 256], F32, tag="mmf")
            for k in range(EK):
                nc.tensor.matmul(
                    pbf[:], ones_b[:], sqf[:, k, ds(li * C, C)],
                    start=(k == 0), stop=(k == EK - 1),
                )
            nc.scalar.activation(
                inv_f[:, ds(li * C, C)], pbf[:], AF.Sqrt,
                bias=dd["eps10"][:], scale=1.0 / float(E),
            )
            nc.vector.reciprocal_approx_fast(
                inv_f[:, ds(li * C, C)], inv_f[:, ds(li * C, C)]
            )
            for k in range(EK):
                nc.vector.tensor_tensor(
                    fin_bf[:, k, ds(li * C, C)],
                    Hs[li][:, k, ds(3 * C, C)],
                    inv_f[:, ds(li * C, C)], OP.mult,
                )

        NB = 512
        n_blocks = (V + NB - 1) // NB  # 63
        for vb in range(n_blocks):
            b0 = vb * NB
            bw = min(NB, V - b0)
            wv_t = wvp.tile([P, EK, NB], BF16, tag=f"wv{vb % 3}", bufs=2,
                            name=f"wv{vb % 3}")
            nc.sync.dma_start(wv_t[:, :, :bw], dd["wtet"][:, :, ds(b0, bw)])
            for m in range(4):
                pl = psum.tile([P, 512], F32, tag="mm", name="pl")
                for k in range(EK):
                    nc.tensor.matmul(
                        pl[:, :bw], fin_bf[:, k, ts(m, P)],
                        wv_t[:, k, :bw],
                        start=(k == 0), stop=(k == EK - 1),
                    )
                ot = work.tile([P, 512], BF16, tag="ot")
                nc.vector.tensor_copy(ot[:, :bw], pl[:, :bw])
                nc.sync.dma_start(
                    out_d[:, m, ds(b0, bw)], ot[:, :bw]
                )


def _emit(nc):
    dd = {
        "xe": nc.dram_tensor("xe", (P, EK, T), F32, kind="ExternalInput"),
        "xnb": nc.dram_tensor("xnb", (P, EK, T), BF16, kind="ExternalInput"),
        "w1t": nc.dram_tensor("w1t", (L, P, EK, ED), BF16, kind="ExternalInput"),
        "eb": nc.dram_tensor("eb", (L, P, EDK), F32, kind="ExternalInput"),
        "fwt": nc.dram_tensor("fwt", (L, P, PK, E), BF16, kind="ExternalInput"),
        "fb": nc.dram_tensor("fb", (L, P, EK), F32, kind="ExternalInput"),
        "mt": nc.dram_tensor("mt", (L, TLK, P, TL), BF16, kind="ExternalInput"),
        "wtet": nc.dram_tensor("wtet", (P, EK, V), BF16, kind="ExternalInput"),
        "oh": nc.dram_tensor("oh", (P, 2), F32, kind="ExternalInput"),
        "ci": nc.dram_tensor("ci", (1, 2), I32, kind="ExternalInput"),
        "nselcol": nc.dram_tensor("nselcol", (N, N), F32, kind="ExternalInput"),
        "nselrow": nc.dram_tensor("nselrow", (N, 1), F32, kind="ExternalInput"),
        "iota8": nc.dram_tensor("iota8", (N, N), F32, kind="ExternalInput"),
    }
    out_d = nc.dram_tensor("out", (P, 2 * C // P, V), BF16, kind="ExternalOutput")

    with tile.TileContext(nc) as tc:
        with tc.tile_pool(name="persist", bufs=1) as persist:
            ident_f = persist.tile([P, P], F32)
            make_identity(nc, ident_f[:])
            ident_b = persist.tile([P, P], BF16)
            make_identity(nc, ident_b[:])
            ones_b = persist.tile([P, P], BF16)
            nc.vector.memset(ones_b[:], 1.0)
            ones_row = persist.tile([1, P], F32)
            nc.vector.memset(ones_row[:], 1.0)
            ones_col = persist.tile([P, 1], F32)
            nc.vector.memset(ones_col[:], 1.0)
            dd["ones_col"] = ones_col
            eps10 = persist.tile([P, 1], F32)
            nc.vector.memset(eps10[:], 1.0e-10)
            eps8 = persist.tile([P, 1], F32)
            nc.vector.memset(eps8[:], 1.0e-8)
            dd["eps10"] = eps10
            dd["eps8"] = eps8
            H0 = persist.tile([P, EK, TL], F32)
            H1 = persist.tile([P, EK, TL], F32)
            Hs = (H0, H1)

            _phase_a(nc, tc, Hs, ident_f, ones_row, dd)
            _layers(nc, tc, Hs, ident_f, ident_b, ones_b, ones_row, dd)
            _logits(nc, tc, Hs, ones_b, dd, out_d)

    return nc


_CACHE = {}


def _get_compiled():
    if "nc" not in _CACHE:
        if LDWOPT:
            from concourse.compiler_utils import (
                get_compiler_flags,
                set_compiler_flags,
            )

            flags = get_compiler_flags()
            new_flags = []
            for f in flags:
                if f.startswith("--internal-backend-options="):
                    f = f.replace("--enable-ldw-opt=false", "--enable-ldw-opt=true")
                new_flags.append(f)
            set_compiler_flags(new_flags)
        nc = bacc.Bacc("TRN2", debug=False, num_devices=8)
        _emit(nc)
        nc.compile()
        _CACHE["nc"] = nc
    return _CACHE["nc"]


def _prep_host(inputs):
    wte = np.asarray(inputs["wte"], np.float32)
    rms_w = np.asarray(inputs["rms_w"], np.float32)
    enr_w = np.asarray(inputs["enr_w"], np.float32)
    enr_b = np.asarray(inputs["enr_b"], np.float32)
    spatial = np.asarray(inputs["spatial"], np.float32)
    fus_w = np.asarray(inputs["fus_w"], np.float32)
    fus_b = np.asarray(inputs["fus_b"], np.float32)
    lnf_w = np.asarray(inputs["lnf_w"], np.float32)

    bf = ml_dtypes.bfloat16
    w1 = enr_w * rms_w[:, None, :]  # fold rms weight
    w1t = np.ascontiguousarray(
        w1.transpose(0, 2, 1).reshape(L, EK, P, ED).transpose(0, 2, 1, 3)
    ).astype(bf)
    eb = np.ascontiguousarray(
        enr_b.reshape(L, EDK, P).transpose(0, 2, 1)
    ).astype(np.float32)
    fwt = np.ascontiguousarray(
        fus_w.transpose(0, 2, 1).reshape(L, PK, P, E).transpose(0, 2, 1, 3)
    ).astype(bf)
    fb = np.ascontiguousarray(
        fus_b.reshape(L, EK, P).transpose(0, 2, 1)
    ).astype(np.float32)
    mt = np.stack([np.tril(spatial[l]).T for l in range(L)])
    mt = np.ascontiguousarray(mt.reshape(L, TLK, P, TL)).astype(bf)
    wtet = np.ascontiguousarray(
        (wte * lnf_w[None, :]).T.reshape(EK, P, V).transpose(1, 0, 2)
    ).astype(bf)

    nselcol = np.zeros((N, N), np.float32)
    nselrow = np.zeros((N, 1), np.float32)
    for i in range(N):
        n_sel = min(i, 3)
        if n_sel > 0:
            nselcol[i, n_sel - 1] = 1.0
        nselrow[i, 0] = float(2 - n_sel)
    iota8 = np.broadcast_to(np.arange(N, dtype=np.float32)[None, :], (N, N)).copy()

    # embedding gather + E-major layout per batch (+ cosine-normalized bf16)
    ids = np.asarray(inputs["input_ids"], np.int32)
    xes, xnbs = [], []
    for b in range(B):
        x = wte[ids[b]]  # [T, E]
        xe = np.ascontiguousarray(x.T.reshape(EK, P, T).transpose(1, 0, 2))
        xn = x / (np.linalg.norm(x, axis=-1, keepdims=True) + 1e-8)
        xnb = np.ascontiguousarray(
            xn.T.reshape(EK, P, T).transpose(1, 0, 2)
        ).astype(bf)
        xes.append(xe.astype(np.float32))
        xnbs.append(xnb)

    return dict(
        w1t=w1t, eb=eb, fwt=fwt, fb=fb, mt=mt, wtet=wtet,
        nselcol=nselcol, nselrow=nselrow, iota8=iota8,
        _xes=xes, _xnbs=xnbs,
    )


def _make_in_maps(inputs, shared):
    sh = {k: v for k, v in shared.items() if not k.startswith("_")}
    in_maps = []
    for c in range(8):
        b = c // 4
        i0 = 2 * (c % 4)
        oh = np.zeros((P, 2), np.float32)
        oh[i0, 0] = 1.0
        oh[i0 + 1, 1] = 1.0
        ci = np.array([[i0, i0 + 1]], np.int32)
        in_maps.append({
            **sh, "xe": shared["_xes"][b], "xnb": shared["_xnbs"][b],
            "oh": oh, "ci": ci,
        })
    return in_maps


def kernel(**inputs):
    shared = _prep_host(inputs)
    nc = _get_compiled()
    in_maps = _make_in_maps(inputs, shared)

    res = run_bass_kernel_spmd(nc, in_maps, core_ids=list(range(8)))
    outs = [r["out"] for r in res.results]  # each [P, 4, V] bf16
    full = np.stack(
        [o.astype(np.float32).transpose(1, 0, 2).reshape(2 * C, V) for o in outs]
    ).reshape(B, 4, 2 * C, V).reshape(B, T, V)
    return full
